# revision 39
# baseline (speedup 1.0000x reference)
"""Trainium2 Bass kernel for nn_ConstGCN.

Math note: in the reference, the attention score s[b,i] is constant along
the softmax axis j, and softmax is shift-invariant, so
p = softmax(s + mask) = softmax(mask) and p.sum(axis=2) == 1 (to ~1e-6 in
f32).  The output therefore collapses to

    out = relu(text + mean_k(emb_table[const_labels[...,k]]) @ fc_W.T + fc_b)

which depends on neither const_mat nor attn_W/attn_b.  The embedding + fc
fuse into a single table M2 = (emb_table @ fc_W.T)/8, so

    out[b,l,:] = relu(text[b,l,:] + sum_k M2[labels[b,l,k], :] + fc_b)

Input marshalling on host: the integer labels [pos, 8] are re-encoded as
per-position class-count vectors (np.bincount; counts in 0..8 are exact in
fp8e4m3), shipped transposed as [class, pos] so the device consumes them
directly as the matmul stationary.  Class row 100 is constant 1 and M2 row
100 = fc_b, folding the bias into the matmul; rows 101..127 are zero.
text is shipped as bf16, the output as fp16 (tolerance is 2e-2; these add
~4e-3).

On device (per core, data-parallel over batch: 2 of 16 batches = 4096
positions, in 4 super-chunks of 1024 positions = 512 KiB DMA transfers,
each computed as 2 sub-chunks of 512 positions):
  - PE: identity matmuls stream the text sub-chunk into PSUM (start=True),
    then four fp8xbf16 matmuls accumulate counts.T @ M2 on top
  - relu + cast f32->fp16 evicts PSUM, split between ACT and DVE
  - DMA: text in alternates sync-HWDGE / gpsimd-SWDGE, out alternates
    scalar-HWDGE / gpsimd-SWDGE so both directions stream concurrently;
    constants load on the scalar ring so text starts at t=0
const_mat (256 MiB) is never read.
"""

import numpy as np
import ml_dtypes

B, L, D = 16, 2048, 256
CN, K = 100, 8
NCLS = 128         # 100 label classes + bias class 100 (M2 row 100 = fc_b)
NCORES = 8
POS = (B // NCORES) * L          # 4096 positions per core
CHUNK = 512                      # positions per compute sub-chunk
NCHUNK = POS // CHUNK            # 8
Q = CHUNK // 128                 # 4 position-groups of 128 per sub-chunk
NSUP = 4                         # DMA super-chunks (1024 positions each)
SUB = NCHUNK // NSUP             # sub-chunks per super-chunk (2)

_compiled = None


def _build():
    import concourse.bacc as bacc
    import concourse.mybir as mybir
    from concourse.tile import TileContext

    f32 = mybir.dt.float32
    fp16 = mybir.dt.float16
    bf16 = mybir.dt.bfloat16
    fp8 = mybir.dt.float8e4

    nc = bacc.Bacc("TRN2", target_bir_lowering=False)

    text_d = nc.dram_tensor("text", [128, NCHUNK * Q * D], bf16,
                            kind="ExternalInput")
    # 128 class rows (101..127 zero-padded): odd partition counts make
    # the DMA fall into a slow descriptor path, so keep all 128
    ct_d = nc.dram_tensor("ct", [128, NCHUNK * Q * 128], fp8,
                          kind="ExternalInput")
    out_d = nc.dram_tensor("out", [128, NCHUNK * Q * D], fp16,
                           kind="ExternalOutput")

    # identity and m2 travel as one [128, 128+256] constant block
    cm_d = nc.dram_tensor("cm", [128, 128 + D], bf16, kind="ExternalInput")

    text_v = text_d
    out_v = out_d
    SUPW = SUB * Q * D               # free words per super-chunk (2048)

    with TileContext(nc) as tc:
        with (
            tc.tile_pool(name="const", bufs=1) as cpool,
            tc.tile_pool(name="in", bufs=NSUP) as ipool,
            tc.tile_pool(name="res", bufs=NSUP) as rpool,
            tc.tile_pool(name="ps", bufs=3, space="PSUM") as pst,
            tc.tile_pool(name="psw", bufs=1, space="PSUM") as psw,
        ):
            # Ring discipline: DMAs sharing a ring drain round-robin, so a
            # transfer's completion sem fires only once everything queued
            # with it drains.  Pair each super-chunk's ct slice with its
            # text on one ring; text3 is gated by ipool buffer reuse so it
            # does not pollute wave 1 on the sync ring.
            #   sync:   ct0, text0, (text3), out3
            #   gpsimd: ct1, ct3, text1, out2
            #   scalar: ident, m2, ct2, text2, out0, out1
            cm_sb = cpool.tile([128, 128 + D], bf16)
            nc.sync.dma_start(out=cm_sb[:, :], in_=cm_d[:, :])
            ident_sb = cm_sb[:, :128]
            m2_sb = cm_sb[:, 128:]
            ct_sb = cpool.tile([128, NCHUNK * Q * 128], fp8)
            ct_v = ct_sb.rearrange("p (n x) -> p n x", n=NCHUNK)

            ct_q = [nc.sync, nc.gpsimd, nc.scalar, nc.gpsimd]
            text_q = [nc.sync, nc.gpsimd, nc.scalar, nc.sync]
            out_q = [nc.gpsimd, nc.gpsimd, nc.sync, nc.scalar]
            CTW = NCHUNK * Q * 128 // NSUP
            for s in range(NSUP):
                ct_q[s].dma_start(out=ct_sb[:, s * CTW:(s + 1) * CTW],
                                  in_=ct_d[:, s * CTW:(s + 1) * CTW])
            text_ts = []
            for s in range(NSUP):
                text_t = ipool.tile([128, SUPW], bf16, tag="text")
                text_q[s].dma_start(out=text_t[:, :],
                                    in_=text_v[:, s * SUPW:(s + 1) * SUPW])
                text_ts.append(text_t)

            # dummy matmuls while inputs stream in: keeps the PE busy so
            # the HAM clock gate opens (1.2 -> 2.4 GHz) before real work
            warm = psw.tile([128, 128], f32)
            for _ in range(12):
                nc.tensor.matmul(warm[:, :], lhsT=ident_sb, rhs=ident_sb,
                                 start=True, stop=True)

            relu = mybir.ActivationFunctionType.Relu
            for s in range(NSUP):
                text_t = text_ts[s]
                res = rpool.tile([128, SUB * Q * D], fp16, tag="res")
                for u in range(SUB):
                    n = s * SUB + u
                    r0 = u * Q * D
                    acc = pst.tile([128, Q * D], f32, tag="acc")
                    if u == 0:
                        # text into PSUM via identity matmuls, fused evict
                        for h in range(2):
                            nc.tensor.matmul(acc[:, h * 512:(h + 1) * 512],
                                             lhsT=ident_sb[:, :],
                                             rhs=text_t[:, r0 + h * 512:
                                                        r0 + (h + 1) * 512],
                                             start=True, stop=False)
                        for q in range(Q):
                            nc.tensor.matmul(
                                acc[:, q * D:(q + 1) * D],
                                lhsT=ct_v[:, n, q * 128:(q + 1) * 128],
                                rhs=m2_sb[:, :],
                                start=False, stop=True,
                            )
                        for h in range(2):
                            nc.scalar.activation(
                                res[:, r0 + h * 512:r0 + (h + 1) * 512],
                                acc[:, h * 512:(h + 1) * 512], relu)
                    else:
                        # E only on PE; DVE adds text, relu split ACT/DVE
                        for q in range(Q):
                            nc.tensor.matmul(
                                acc[:, q * D:(q + 1) * D],
                                lhsT=ct_v[:, n, q * 128:(q + 1) * 128],
                                rhs=m2_sb[:, :],
                                start=True, stop=True,
                            )
                        for h in range(2):
                            nc.vector.tensor_tensor(
                                out=res[:, r0 + h * 512:r0 + (h + 1) * 512],
                                in0=text_t[:, r0 + h * 512:r0 + (h + 1) * 512],
                                in1=acc[:, h * 512:(h + 1) * 512],
                                op=mybir.AluOpType.add)
                        for h in range(2):
                            # fp16 in-place relu runs in DVE 4x mode
                            nc.vector.tensor_scalar_max(
                                out=res[:, r0 + h * 512:r0 + (h + 1) * 512],
                                in0=res[:, r0 + h * 512:r0 + (h + 1) * 512],
                                scalar1=0.0)
                out_q[s].dma_start(out=out_v[:, s * SUPW:(s + 1) * SUPW],
                                   in_=res[:, :])

    nc.finalize()
    return nc


def _get_compiled():
    global _compiled
    if _compiled is None:
        _compiled = _build()
    return _compiled


def _host_prep(text, const_labels, emb_table, fc_W, fc_b):
    """Marshal full inputs -> per-core in_maps."""
    # fused gather table: row c (c<CN) = (emb_table @ fc_W.T)[c]/8,
    # row 100 = fc_b (count row 100 is constant 1), rows 101..127 zero
    m2 = np.zeros((NCLS, D), dtype=np.float64)
    m2[:CN] = emb_table.astype(np.float64) @ fc_W.T.astype(np.float64) * 0.125
    m2[CN] = fc_b
    cm = np.concatenate([np.eye(128, dtype=np.float64), m2], axis=1)
    cm = np.ascontiguousarray(cm.astype(ml_dtypes.bfloat16))

    # label -> count-vector encoding (counts 0..8, exact in fp8e4m3)
    lab = np.ascontiguousarray(const_labels.reshape(B * L, K)).astype(np.int64)
    ids = (np.arange(B * L, dtype=np.int64) * CN)[:, None] + lab
    counts = np.bincount(ids.ravel(), minlength=B * L * CN).reshape(B * L, CN)
    # layout per core: [class, n, q*128 + p] with pos = n*512 + p*4 + q;
    # row 100 = all-ones bias row, 101..127 zero
    cc = counts.reshape(NCORES, NCHUNK, 128, Q, CN)
    ct = np.zeros((NCORES, NCLS, NCHUNK, Q, 128), dtype=np.float32)
    ct[:, :CN] = cc.transpose(0, 4, 1, 3, 2)
    ct[:, CN] = 1.0
    ct = ct.reshape(NCORES, NCLS, NCHUNK * Q * 128).astype(
        ml_dtypes.float8_e4m3fn)

    text16 = np.ascontiguousarray(text.reshape(B * L, D)).astype(
        ml_dtypes.bfloat16)
    # partition-major per core: [128, n*q*d], pos = n*512 + p*4 + q
    text16 = text16.reshape(NCORES, NCHUNK, 128, Q * D).transpose(0, 2, 1, 3)
    text16 = text16.reshape(NCORES, 128, NCHUNK * Q * D)

    in_maps = []
    for c in range(NCORES):
        in_maps.append({
            "text": np.ascontiguousarray(text16[c]),
            "ct": np.ascontiguousarray(ct[c]),
            "cm": cm,
        })
    return in_maps


def kernel(text, const_mat, const_labels, emb_table, attn_W, attn_b,
           fc_W, fc_b):
    from concourse.bass_utils import run_bass_kernel_spmd

    text = np.asarray(text, dtype=np.float32)
    const_labels = np.asarray(const_labels)
    emb_table = np.asarray(emb_table, dtype=np.float32)
    fc_W = np.asarray(fc_W, dtype=np.float32)
    fc_b = np.asarray(fc_b, dtype=np.float32)

    in_maps = _host_prep(text, const_labels, emb_table, fc_W, fc_b)
    nc = _get_compiled()
    r = run_bass_kernel_spmd(nc, in_maps, core_ids=list(range(NCORES)))
    out = np.stack([r.results[c]["out"] for c in range(NCORES)], axis=0)
    # [core, p, (n q d)] -> [core, n, p, q, d]; position = n*512 + p*4 + q
    out = out.reshape(NCORES, 128, NCHUNK, Q * D).transpose(0, 2, 1, 3)
    return out.astype(np.float32).reshape(B, L, D)


# revision 43
# speedup vs baseline: 1.0932x; 1.0932x over previous
"""Trainium2 Bass kernel for nn_ConstGCN.

Math note: in the reference, the attention score s[b,i] is constant along
the softmax axis j, and softmax is shift-invariant, so
p = softmax(s + mask) = softmax(mask) and p.sum(axis=2) == 1 (to ~1e-6 in
f32).  The output therefore collapses to

    out = relu(text + mean_k(emb_table[const_labels[...,k]]) @ fc_W.T + fc_b)

which depends on neither const_mat nor attn_W/attn_b.  The embedding + fc
fuse into a single table M2 = (emb_table @ fc_W.T)/8, so

    out[b,l,:] = relu(text[b,l,:] + sum_k M2[labels[b,l,k], :] + fc_b)

Input marshalling on host: the integer labels [pos, 8] are re-encoded as
per-position class-count vectors (np.bincount; counts in 0..8 are exact in
fp8e4m3), shipped transposed as [class, pos] so the device consumes them
directly as the matmul stationary.  Class row 100 is constant 1 and M2 row
100 = fc_b, folding the bias into the matmul; rows 101..127 are zero.
text is shipped as bf16, the output as fp16 (tolerance is 2e-2; these add
~4e-3).

On device (per core, data-parallel over batch: 2 of 16 batches = 4096
positions, in 4 super-chunks of 1024 positions = 512 KiB DMA transfers,
each computed as 2 sub-chunks of 512 positions):
  - PE: identity matmuls stream the text sub-chunk into PSUM (start=True),
    then four fp8xbf16 matmuls accumulate counts.T @ M2 on top
  - relu + cast f32->fp16 evicts PSUM, split between ACT and DVE
  - DMA: text in alternates sync-HWDGE / gpsimd-SWDGE, out alternates
    scalar-HWDGE / gpsimd-SWDGE so both directions stream concurrently;
    constants load on the scalar ring so text starts at t=0
const_mat (256 MiB) is never read.
"""

import numpy as np
import ml_dtypes

B, L, D = 16, 2048, 256
CN, K = 100, 8
NCLS = 128         # 100 label classes + bias class 100 (M2 row 100 = fc_b)
NCORES = 8
POS = (B // NCORES) * L          # 4096 positions per core
CHUNK = 512                      # positions per compute sub-chunk
NCHUNK = POS // CHUNK            # 8
Q = CHUNK // 128                 # 4 position-groups of 128 per sub-chunk
NSUP = 4                         # DMA super-chunks (1024 positions each)
SUB = NCHUNK // NSUP             # sub-chunks per super-chunk (2)

_compiled = None


def _build():
    import concourse.bacc as bacc
    import concourse.mybir as mybir
    from concourse.tile import TileContext

    f32 = mybir.dt.float32
    fp16 = mybir.dt.float16
    bf16 = mybir.dt.bfloat16
    fp8 = mybir.dt.float8e4

    nc = bacc.Bacc("TRN2", target_bir_lowering=False)

    text_d = nc.dram_tensor("text", [128, NCHUNK * Q * D], bf16,
                            kind="ExternalInput")
    # 128 class rows (101..127 zero-padded): odd partition counts make
    # the DMA fall into a slow descriptor path, so keep all 128
    ct_d = nc.dram_tensor("ct", [128, NCHUNK * Q * 128], fp8,
                          kind="ExternalInput")
    out_d = nc.dram_tensor("out", [128, NCHUNK * Q * D], fp16,
                           kind="ExternalOutput")

    # identity and m2 travel as one [128, 128+256] constant block
    cm_d = nc.dram_tensor("cm", [128, 128 + D], bf16, kind="ExternalInput")

    text_v = text_d
    out_v = out_d
    SUPW = SUB * Q * D               # free words per super-chunk (2048)

    with TileContext(nc) as tc:
        with (
            tc.tile_pool(name="const", bufs=1) as cpool,
            tc.tile_pool(name="in", bufs=3) as ipool,
            tc.tile_pool(name="res", bufs=NSUP) as rpool,
            tc.tile_pool(name="ps", bufs=3, space="PSUM") as pst,
            tc.tile_pool(name="psw", bufs=1, space="PSUM") as psw,
        ):
            # Ring discipline: DMAs sharing a ring drain round-robin, so a
            # transfer's completion sem fires only once everything queued
            # with it drains.  Pair each super-chunk's ct slice with its
            # text on one ring; text3 is gated by ipool buffer reuse so it
            # does not pollute wave 1 on the sync ring.
            #   sync:   ct0, text0, (text3), out3
            #   gpsimd: ct1, ct3, text1, out2
            #   scalar: ident, m2, ct2, text2, out0, out1
            cm_sb = cpool.tile([128, 128 + D], bf16)
            nc.scalar.dma_start(out=cm_sb[:, :], in_=cm_d[:, :])
            ident_sb = cm_sb[:, :128]
            m2_sb = cm_sb[:, 128:]
            ct_sb = cpool.tile([128, NCHUNK * Q * 128], fp8)
            ct_v = ct_sb.rearrange("p (n x) -> p n x", n=NCHUNK)

            ct_q = [nc.sync, nc.gpsimd, nc.scalar, nc.gpsimd]
            text_q = [nc.sync, nc.gpsimd, nc.scalar, nc.sync]
            out_q = [nc.gpsimd, nc.gpsimd, nc.sync, nc.sync]
            CTW = NCHUNK * Q * 128 // NSUP
            for s in range(NSUP):
                ct_q[s].dma_start(out=ct_sb[:, s * CTW:(s + 1) * CTW],
                                  in_=ct_d[:, s * CTW:(s + 1) * CTW])
            text_ts = []
            for s in range(NSUP):
                text_t = ipool.tile([128, SUPW], bf16, tag="text")
                text_q[s].dma_start(out=text_t[:, :],
                                    in_=text_v[:, s * SUPW:(s + 1) * SUPW])
                text_ts.append(text_t)

            # dummy matmuls while inputs stream in: keeps the PE busy so
            # the HAM clock gate opens (1.2 -> 2.4 GHz) before real work
            warm = psw.tile([128, 128], f32)
            for _ in range(24):
                nc.tensor.matmul(warm[:, :], lhsT=ident_sb, rhs=ident_sb,
                                 start=True, stop=True)

            relu = mybir.ActivationFunctionType.Relu
            for s in range(NSUP):
                text_t = text_ts[s]
                res = rpool.tile([128, SUB * Q * D], fp16, tag="res")
                for u in range(SUB):
                    n = s * SUB + u
                    r0 = u * Q * D
                    acc = pst.tile([128, Q * D], f32, tag="acc")
                    if u == 0:
                        # text into PSUM via identity matmuls, fused evict
                        for h in range(2):
                            nc.tensor.matmul(acc[:, h * 512:(h + 1) * 512],
                                             lhsT=ident_sb[:, :],
                                             rhs=text_t[:, r0 + h * 512:
                                                        r0 + (h + 1) * 512],
                                             start=True, stop=False)
                        for q in range(Q):
                            nc.tensor.matmul(
                                acc[:, q * D:(q + 1) * D],
                                lhsT=ct_v[:, n, q * 128:(q + 1) * 128],
                                rhs=m2_sb[:, :],
                                start=False, stop=True,
                            )
                        for h in range(2):
                            nc.scalar.activation(
                                res[:, r0 + h * 512:r0 + (h + 1) * 512],
                                acc[:, h * 512:(h + 1) * 512], relu)
                    else:
                        # E only on PE; DVE adds text, relu split ACT/DVE
                        for q in range(Q):
                            nc.tensor.matmul(
                                acc[:, q * D:(q + 1) * D],
                                lhsT=ct_v[:, n, q * 128:(q + 1) * 128],
                                rhs=m2_sb[:, :],
                                start=True, stop=True,
                            )
                        for h in range(2):
                            nc.vector.tensor_tensor(
                                out=res[:, r0 + h * 512:r0 + (h + 1) * 512],
                                in0=text_t[:, r0 + h * 512:r0 + (h + 1) * 512],
                                in1=acc[:, h * 512:(h + 1) * 512],
                                op=mybir.AluOpType.add)
                        for h in range(2):
                            # fp16 in-place relu runs in DVE 4x mode
                            nc.vector.tensor_scalar_max(
                                out=res[:, r0 + h * 512:r0 + (h + 1) * 512],
                                in0=res[:, r0 + h * 512:r0 + (h + 1) * 512],
                                scalar1=0.0)
                out_q[s].dma_start(out=out_v[:, s * SUPW:(s + 1) * SUPW],
                                   in_=res[:, :])

    nc.finalize()
    return nc


def _get_compiled():
    global _compiled
    if _compiled is None:
        _compiled = _build()
    return _compiled


def _host_prep(text, const_labels, emb_table, fc_W, fc_b):
    """Marshal full inputs -> per-core in_maps."""
    # fused gather table: row c (c<CN) = (emb_table @ fc_W.T)[c]/8,
    # row 100 = fc_b (count row 100 is constant 1), rows 101..127 zero
    m2 = np.zeros((NCLS, D), dtype=np.float64)
    m2[:CN] = emb_table.astype(np.float64) @ fc_W.T.astype(np.float64) * 0.125
    m2[CN] = fc_b
    cm = np.concatenate([np.eye(128, dtype=np.float64), m2], axis=1)
    cm = np.ascontiguousarray(cm.astype(ml_dtypes.bfloat16))

    # label -> count-vector encoding (counts 0..8, exact in fp8e4m3)
    lab = np.ascontiguousarray(const_labels.reshape(B * L, K)).astype(np.int64)
    ids = (np.arange(B * L, dtype=np.int64) * CN)[:, None] + lab
    counts = np.bincount(ids.ravel(), minlength=B * L * CN).reshape(B * L, CN)
    # layout per core: [class, n, q*128 + p] with pos = n*512 + p*4 + q;
    # row 100 = all-ones bias row, 101..127 zero
    cc = counts.reshape(NCORES, NCHUNK, 128, Q, CN)
    ct = np.zeros((NCORES, NCLS, NCHUNK, Q, 128), dtype=np.float32)
    ct[:, :CN] = cc.transpose(0, 4, 1, 3, 2)
    ct[:, CN] = 1.0
    ct = ct.reshape(NCORES, NCLS, NCHUNK * Q * 128).astype(
        ml_dtypes.float8_e4m3fn)

    text16 = np.ascontiguousarray(text.reshape(B * L, D)).astype(
        ml_dtypes.bfloat16)
    # partition-major per core: [128, n*q*d], pos = n*512 + p*4 + q
    text16 = text16.reshape(NCORES, NCHUNK, 128, Q * D).transpose(0, 2, 1, 3)
    text16 = text16.reshape(NCORES, 128, NCHUNK * Q * D)

    in_maps = []
    for c in range(NCORES):
        in_maps.append({
            "text": np.ascontiguousarray(text16[c]),
            "ct": np.ascontiguousarray(ct[c]),
            "cm": cm,
        })
    return in_maps


def kernel(text, const_mat, const_labels, emb_table, attn_W, attn_b,
           fc_W, fc_b):
    from concourse.bass_utils import run_bass_kernel_spmd

    text = np.asarray(text, dtype=np.float32)
    const_labels = np.asarray(const_labels)
    emb_table = np.asarray(emb_table, dtype=np.float32)
    fc_W = np.asarray(fc_W, dtype=np.float32)
    fc_b = np.asarray(fc_b, dtype=np.float32)

    in_maps = _host_prep(text, const_labels, emb_table, fc_W, fc_b)
    nc = _get_compiled()
    r = run_bass_kernel_spmd(nc, in_maps, core_ids=list(range(NCORES)))
    out = np.stack([r.results[c]["out"] for c in range(NCORES)], axis=0)
    # [core, p, (n q d)] -> [core, n, p, q, d]; position = n*512 + p*4 + q
    out = out.reshape(NCORES, 128, NCHUNK, Q * D).transpose(0, 2, 1, 3)
    return out.astype(np.float32).reshape(B, L, D)


# revision 44
# speedup vs baseline: 1.1225x; 1.0268x over previous
"""Trainium2 Bass kernel for nn_ConstGCN.

Math note: in the reference, the attention score s[b,i] is constant along
the softmax axis j, and softmax is shift-invariant, so
p = softmax(s + mask) = softmax(mask) and p.sum(axis=2) == 1 (to ~1e-6 in
f32).  The output therefore collapses to

    out = relu(text + mean_k(emb_table[const_labels[...,k]]) @ fc_W.T + fc_b)

which depends on neither const_mat nor attn_W/attn_b.  The embedding + fc
fuse into a single table M2 = (emb_table @ fc_W.T)/8, so

    out[b,l,:] = relu(text[b,l,:] + sum_k M2[labels[b,l,k], :] + fc_b)

Input marshalling on host: the integer labels [pos, 8] are re-encoded as
per-position class-count vectors (np.bincount; counts in 0..8 are exact in
fp8e4m3), shipped transposed as [class, pos] so the device consumes them
directly as the matmul stationary.  Class row 100 is constant 1 and M2 row
100 = fc_b, folding the bias into the matmul; rows 101..127 are zero.
text is shipped as bf16, the output as fp16 (tolerance is 2e-2; these add
~4e-3).

On device (per core, data-parallel over batch: 2 of 16 batches = 4096
positions, in 4 super-chunks of 1024 positions = 512 KiB DMA transfers,
each computed as 2 sub-chunks of 512 positions):
  - PE: identity matmuls stream the text sub-chunk into PSUM (start=True),
    then four fp8xbf16 matmuls accumulate counts.T @ M2 on top
  - relu + cast f32->fp16 evicts PSUM, split between ACT and DVE
  - DMA: text in alternates sync-HWDGE / gpsimd-SWDGE, out alternates
    scalar-HWDGE / gpsimd-SWDGE so both directions stream concurrently;
    constants load on the scalar ring so text starts at t=0
const_mat (256 MiB) is never read.
"""

import numpy as np
import ml_dtypes

B, L, D = 16, 2048, 256
CN, K = 100, 8
NCLS = 128         # 100 label classes + bias class 100 (M2 row 100 = fc_b)
NCORES = 8
POS = (B // NCORES) * L          # 4096 positions per core
CHUNK = 512                      # positions per compute sub-chunk
NCHUNK = POS // CHUNK            # 8
Q = CHUNK // 128                 # 4 position-groups of 128 per sub-chunk
NSUP = 4                         # DMA super-chunks (1024 positions each)
SUB = NCHUNK // NSUP             # sub-chunks per super-chunk (2)

_compiled = None


def _build():
    import concourse.bacc as bacc
    import concourse.mybir as mybir
    from concourse.tile import TileContext

    f32 = mybir.dt.float32
    fp16 = mybir.dt.float16
    bf16 = mybir.dt.bfloat16
    fp8 = mybir.dt.float8e4

    nc = bacc.Bacc("TRN2", target_bir_lowering=False)

    text_d = nc.dram_tensor("text", [128, NCHUNK * Q * D], bf16,
                            kind="ExternalInput")
    # 128 class rows (101..127 zero-padded): odd partition counts make
    # the DMA fall into a slow descriptor path, so keep all 128
    ct_d = nc.dram_tensor("ct", [128, NCHUNK * Q * 128], fp8,
                          kind="ExternalInput")
    out_d = nc.dram_tensor("out", [128, NCHUNK * Q * D], fp16,
                           kind="ExternalOutput")

    # identity and m2 travel as one [128, 128+256] constant block
    cm_d = nc.dram_tensor("cm", [128, 128 + D], bf16, kind="ExternalInput")

    text_v = text_d
    out_v = out_d
    SUPW = SUB * Q * D               # free words per super-chunk (2048)

    with TileContext(nc) as tc:
        with (
            tc.tile_pool(name="const", bufs=1) as cpool,
            tc.tile_pool(name="in", bufs=3) as ipool,
            tc.tile_pool(name="res", bufs=NSUP) as rpool,
            tc.tile_pool(name="ps", bufs=3, space="PSUM") as pst,
            tc.tile_pool(name="psw", bufs=1, space="PSUM") as psw,
        ):
            # Ring discipline: DMAs sharing a ring drain round-robin, so a
            # transfer's completion sem fires only once everything queued
            # with it drains.  Pair each super-chunk's ct slice with its
            # text on one ring; text3 is gated by ipool buffer reuse so it
            # does not pollute wave 1 on the sync ring.
            #   sync:   ct0, text0, (text3), out3
            #   gpsimd: ct1, ct3, text1, out2
            #   scalar: ident, m2, ct2, text2, out0, out1
            cm_sb = cpool.tile([128, 128 + D], bf16)
            nc.scalar.dma_start(out=cm_sb[:, :], in_=cm_d[:, :])
            ident_sb = cm_sb[:, :128]
            m2_sb = cm_sb[:, 128:]
            ct_sb = cpool.tile([128, NCHUNK * Q * 128], fp8)
            ct_v = ct_sb.rearrange("p (n x) -> p n x", n=NCHUNK)

            ct_q = [nc.sync, nc.gpsimd, nc.scalar, nc.gpsimd]
            text_q = [nc.sync, nc.gpsimd, nc.scalar, nc.sync]
            out_q = [nc.gpsimd, nc.gpsimd, nc.sync, nc.scalar]
            CTW = NCHUNK * Q * 128 // NSUP
            for s in range(NSUP):
                ct_q[s].dma_start(out=ct_sb[:, s * CTW:(s + 1) * CTW],
                                  in_=ct_d[:, s * CTW:(s + 1) * CTW])
            text_ts = []
            for s in range(NSUP):
                text_t = ipool.tile([128, SUPW], bf16, tag="text")
                text_q[s].dma_start(out=text_t[:, :],
                                    in_=text_v[:, s * SUPW:(s + 1) * SUPW])
                text_ts.append(text_t)

            # dummy matmuls while inputs stream in: keeps the PE busy so
            # the HAM clock gate opens (1.2 -> 2.4 GHz) before real work
            warm = psw.tile([128, 128], f32)
            for _ in range(24):
                nc.tensor.matmul(warm[:, :], lhsT=ident_sb, rhs=ident_sb,
                                 start=True, stop=True)

            relu = mybir.ActivationFunctionType.Relu
            for s in range(NSUP):
                text_t = text_ts[s]
                res = rpool.tile([128, SUB * Q * D], fp16, tag="res")
                for u in range(SUB):
                    n = s * SUB + u
                    r0 = u * Q * D
                    acc = pst.tile([128, Q * D], f32, tag="acc")
                    if u == 0:
                        # text into PSUM via identity matmuls, fused evict
                        for h in range(2):
                            nc.tensor.matmul(acc[:, h * 512:(h + 1) * 512],
                                             lhsT=ident_sb[:, :],
                                             rhs=text_t[:, r0 + h * 512:
                                                        r0 + (h + 1) * 512],
                                             start=True, stop=False)
                        for q in range(Q):
                            nc.tensor.matmul(
                                acc[:, q * D:(q + 1) * D],
                                lhsT=ct_v[:, n, q * 128:(q + 1) * 128],
                                rhs=m2_sb[:, :],
                                start=False, stop=True,
                            )
                        for h in range(2):
                            nc.scalar.activation(
                                res[:, r0 + h * 512:r0 + (h + 1) * 512],
                                acc[:, h * 512:(h + 1) * 512], relu)
                    else:
                        # E only on PE; DVE adds text, relu split ACT/DVE
                        for q in range(Q):
                            nc.tensor.matmul(
                                acc[:, q * D:(q + 1) * D],
                                lhsT=ct_v[:, n, q * 128:(q + 1) * 128],
                                rhs=m2_sb[:, :],
                                start=True, stop=True,
                            )
                        for h in range(2):
                            nc.vector.tensor_tensor(
                                out=res[:, r0 + h * 512:r0 + (h + 1) * 512],
                                in0=text_t[:, r0 + h * 512:r0 + (h + 1) * 512],
                                in1=acc[:, h * 512:(h + 1) * 512],
                                op=mybir.AluOpType.add)
                        for h in range(2):
                            # fp16 in-place relu runs in DVE 4x mode
                            nc.vector.tensor_scalar_max(
                                out=res[:, r0 + h * 512:r0 + (h + 1) * 512],
                                in0=res[:, r0 + h * 512:r0 + (h + 1) * 512],
                                scalar1=0.0)
                out_q[s].dma_start(out=out_v[:, s * SUPW:(s + 1) * SUPW],
                                   in_=res[:, :])

    nc.finalize()
    return nc


def _get_compiled():
    global _compiled
    if _compiled is None:
        _compiled = _build()
    return _compiled


def _host_prep(text, const_labels, emb_table, fc_W, fc_b):
    """Marshal full inputs -> per-core in_maps."""
    # fused gather table: row c (c<CN) = (emb_table @ fc_W.T)[c]/8,
    # row 100 = fc_b (count row 100 is constant 1), rows 101..127 zero
    m2 = np.zeros((NCLS, D), dtype=np.float64)
    m2[:CN] = emb_table.astype(np.float64) @ fc_W.T.astype(np.float64) * 0.125
    m2[CN] = fc_b
    cm = np.concatenate([np.eye(128, dtype=np.float64), m2], axis=1)
    cm = np.ascontiguousarray(cm.astype(ml_dtypes.bfloat16))

    # label -> count-vector encoding (counts 0..8, exact in fp8e4m3)
    lab = np.ascontiguousarray(const_labels.reshape(B * L, K)).astype(np.int64)
    ids = (np.arange(B * L, dtype=np.int64) * CN)[:, None] + lab
    counts = np.bincount(ids.ravel(), minlength=B * L * CN).reshape(B * L, CN)
    # layout per core: [class, n, q*128 + p] with pos = n*512 + p*4 + q;
    # row 100 = all-ones bias row, 101..127 zero
    cc = counts.reshape(NCORES, NCHUNK, 128, Q, CN)
    ct = np.zeros((NCORES, NCLS, NCHUNK, Q, 128), dtype=np.float32)
    ct[:, :CN] = cc.transpose(0, 4, 1, 3, 2)
    ct[:, CN] = 1.0
    ct = ct.reshape(NCORES, NCLS, NCHUNK * Q * 128).astype(
        ml_dtypes.float8_e4m3fn)

    text16 = np.ascontiguousarray(text.reshape(B * L, D)).astype(
        ml_dtypes.bfloat16)
    # partition-major per core: [128, n*q*d], pos = n*512 + p*4 + q
    text16 = text16.reshape(NCORES, NCHUNK, 128, Q * D).transpose(0, 2, 1, 3)
    text16 = text16.reshape(NCORES, 128, NCHUNK * Q * D)

    in_maps = []
    for c in range(NCORES):
        in_maps.append({
            "text": np.ascontiguousarray(text16[c]),
            "ct": np.ascontiguousarray(ct[c]),
            "cm": cm,
        })
    return in_maps


def kernel(text, const_mat, const_labels, emb_table, attn_W, attn_b,
           fc_W, fc_b):
    from concourse.bass_utils import run_bass_kernel_spmd

    text = np.asarray(text, dtype=np.float32)
    const_labels = np.asarray(const_labels)
    emb_table = np.asarray(emb_table, dtype=np.float32)
    fc_W = np.asarray(fc_W, dtype=np.float32)
    fc_b = np.asarray(fc_b, dtype=np.float32)

    in_maps = _host_prep(text, const_labels, emb_table, fc_W, fc_b)
    nc = _get_compiled()
    r = run_bass_kernel_spmd(nc, in_maps, core_ids=list(range(NCORES)))
    out = np.stack([r.results[c]["out"] for c in range(NCORES)], axis=0)
    # [core, p, (n q d)] -> [core, n, p, q, d]; position = n*512 + p*4 + q
    out = out.reshape(NCORES, 128, NCHUNK, Q * D).transpose(0, 2, 1, 3)
    return out.astype(np.float32).reshape(B, L, D)
